# revision 11
# baseline (speedup 1.0000x reference)
"""LIF spike kernel (T-step leaky integrate-and-fire recurrence) on 8 TRN2 cores.

Reference semantics (per element, thre = tanh(w[c])):
    u_t = TAU * u_{t-1} * (1 - o_{t-1}) + x_t
    o_t = (u_t - thre > 0) ? 1.0 : 0.0

Raw-bass implementation (no Tile — this walrus build allows only one sync
wait per compute instruction, so waits are standalone wait_ge instructions).

Per step, carrying M_t = u_t * (u_t <= thre):
    DVE:  U  = (M * TAU) + X_t          scalar_tensor_tensor (mult, add)
    DVE:  M  = (U <= thre) * U          scalar_tensor_tensor (is_le, mult)
    ACT:  SG = Sign(U - thre)           activation Sign, bias = -tanh(w)
    ACT:  O  = Relu(SG) -> uint8        exact 0/1 spikes
    ACT:  dma o[t] <- O
All products are by 1.0/0.0 masks or by TAU=0.25 (a power of two), and the
compare path matches the reference's (u - thre > 0), so the result is
bit-exact vs the fp32 reference.

Sharding: B=32 split across 8 cores (4 each).  Per-core SBUF layout:
partition p = bp*64 + c (bp = batch pair, c = channel), free f = bf*1024 + hw,
with b = bp*2 + bf.  The host pre-transposes x so each timestep is one
contiguous [128, 2048] fp32 DMA; spikes return as uint8 and are cast on host.
"""

import numpy as np

import concourse.bass as bass
import concourse.mybir as mybir
from concourse.bass_utils import run_bass_kernel_spmd

TAU = 0.25
T, B, C, H, W = 16, 32, 64, 32, 32
N_CORES = 8
B_PER = B // N_CORES  # 4
HWF = H * W  # 1024
P = 128  # partitions: 2 batch-pairs x 64 channels
FD = (B_PER // 2) * HWF  # 2048 free-dim elements per partition per step

XS = 3  # X double-buffer slots
US = 2  # U slots
OS = 8  # O slots

_cache = {}
last_results = None  # BassKernelResults of the most recent run (for test harness)


def _build_nc():
    nc = bass.Bass("TRN2", target_bir_lowering=False, debug=False, num_devices=N_CORES)
    f32 = mybir.dt.float32
    u8 = mybir.dt.uint8
    x_d = nc.dram_tensor("x", [T, P, FD], f32, kind="ExternalInput").ap()
    w_d = nc.dram_tensor("w", [P, 1], f32, kind="ExternalInput").ap()
    o_d = nc.dram_tensor("o", [T, P, FD], u8, kind="ExternalOutput").ap()

    AT = mybir.AluOpType
    AF = mybir.ActivationFunctionType

    X = nc.alloc_sbuf_tensor("Xb", [P, XS * FD], f32).ap()
    U = nc.alloc_sbuf_tensor("Ub", [P, US * FD], f32).ap()
    M = nc.alloc_sbuf_tensor("Mb", [P, FD], f32).ap()
    SG = nc.alloc_sbuf_tensor("SGb", [P, FD], f32).ap()
    O = nc.alloc_sbuf_tensor("Ob", [P, OS * FD], u8).ap()
    WT = nc.alloc_sbuf_tensor("WTb", [P, 1], f32).ap()
    NT = nc.alloc_sbuf_tensor("NTb", [P, 1], f32).ap()  # -tanh(w)
    TH = nc.alloc_sbuf_tensor("THb", [P, 1], f32).ap()  # +tanh(w)

    def xsl(t):
        return X[:, (t % XS) * FD : (t % XS + 1) * FD]

    def usl(t):
        return U[:, (t % US) * FD : (t % US + 1) * FD]

    def osl(t):
        return O[:, (t % OS) * FD : (t % OS + 1) * FD]

    import contextlib

    with contextlib.ExitStack() as st:
        block = st.enter_context(nc.Block())
        dve = st.enter_context(nc.semaphore("dve"))
        act = st.enter_context(nc.semaphore("act"))
        dw = st.enter_context(nc.semaphore("dw"))
        # one sem per SBUF slot -> never more than one outstanding inc per sem,
        # so count-based waits are unambiguous under out-of-order DMA completion
        dx = [st.enter_context(nc.semaphore(f"dx{i}")) for i in range(XS)]
        do = [st.enter_context(nc.semaphore(f"do{i}")) for i in range(OS)]

        @block.sync
        def _(sp):
            sp.dma_start(out=WT, in_=w_d).then_inc(dw, 16)
            for t in range(T):
                if t >= XS:
                    sp.wait_ge(dve, t - XS + 1)  # STT2(t-XS) read its X slot
                sp.dma_start(out=xsl(t), in_=x_d[t]).then_inc(dx[t % XS], 16)

        @block.scalar
        def _(ac):
            ac.wait_ge(dw, 16)
            ac.activation(NT, WT, AF.Tanh, scale=-1.0)  # tanh odd: -tanh(w)
            ac.activation(TH, WT, AF.Tanh).then_inc(act, 1)
            ac.drain()
            for t in range(T):
                ac.wait_ge(dve, t + 1)  # U(t) ready
                ac.activation(SG, usl(t), AF.Sign, bias=NT).then_inc(act, 1)
                if t >= OS:
                    ac.wait_ge(do[t % OS], 16 * (t // OS))  # O slot drained
                ac.drain()
                ac.activation(osl(t), SG, AF.Relu)
                ac.drain()
                ac.dma_start(out=o_d[t], in_=osl(t)).then_inc(do[t % OS], 16)
            for i in range(OS):
                n_dmas = len([t for t in range(T) if t % OS == i])
                ac.wait_ge(do[i], 16 * n_dmas)

        @block.vector
        def _(dv):
            dv.wait_ge(act, 1)  # thre ready
            dv.memset(M, 0.0)
            dv.drain()
            for t in range(T):
                dv.wait_ge(dx[t % XS], 16 * (t // XS + 1))  # X(t) loaded
                if t >= US:
                    dv.wait_ge(act, t)  # Sign(t-US) read its U slot
                dv.scalar_tensor_tensor(
                    usl(t), M, TAU, xsl(t), AT.mult, AT.add
                ).then_inc(dve, 1)
                dv.drain()
                dv.scalar_tensor_tensor(M, usl(t), TH, usl(t), AT.is_le, AT.mult)
                dv.drain()

    return nc


def _get_nc():
    if "nc" not in _cache:
        _cache["nc"] = _build_nc()
    return _cache["nc"]


def _shard_x(x):
    """x [T,B,C,H,W] fp32 -> list of 8 contiguous [T,128,2048] arrays."""
    xf = x.reshape(T, B, C, HWF)
    shards = []
    for i in range(N_CORES):
        xc = xf[:, i * B_PER : (i + 1) * B_PER]  # [T,4,C,1024]
        xc = xc.reshape(T, 2, 2, C, HWF).transpose(0, 1, 3, 2, 4)  # t,bp,c,bf,f
        shards.append(np.ascontiguousarray(xc).reshape(T, P, FD))
    return shards


def _unshard_o(per_core):
    """list of 8 [T,128,2048] uint8 -> [T,B,C,H,W] fp32."""
    outs = []
    for oc in per_core:
        oc = oc.reshape(T, 2, C, 2, HWF).transpose(0, 1, 3, 2, 4)  # t,bp,bf,c,f
        outs.append(oc.reshape(T, B_PER, C, H, W))
    return np.concatenate(outs, axis=1).astype(np.float32)


def kernel(x, w):
    global last_results
    x = np.ascontiguousarray(np.asarray(x), dtype=np.float32)
    w = np.tile(np.asarray(w, dtype=np.float32).reshape(64, 1), (2, 1))  # [128,1]

    nc = _get_nc()
    shards = _shard_x(x)
    in_maps = [{"x": shards[i], "w": w} for i in range(N_CORES)]
    last_results = run_bass_kernel_spmd(nc, in_maps, core_ids=list(range(N_CORES)))
    return _unshard_o([last_results.results[i]["o"] for i in range(N_CORES)])
